# revision 6
# baseline (speedup 1.0000x reference)
"""Trainium2 Bass kernel for 16-head MultiHeadAttention (B=2, S=2048, D=1024, f32).

Sharding: 8 cores = 2 (batch) x 4 (head groups of 4 heads).
Each core gets col-shards of Wq/Wk/Wv ([1024,256]) + a row-shard of Wo
([256,1024]), computes a full [2048,1024] partial output; the host sums the
8 partials (4 per batch element) into [2,2048,1024].

All device data is bf16 (f32 accumulation in PSUM); the host converts inputs
and upconverts the bf16 partial outputs.

The ACT engine (exp over the 4 x 2048 x 2048 score matrix: 128 instructions,
~1.15us each) is the pacing engine; everything else is arranged to hide
behind it:
  - single PSUM pool with two tag-rings ("sc" 2x2 banks for scores+phase-A
    projection pairs, "acc" 4x1 banks for AV/transpose/Wo) so there is no
    pool-boundary barrier between projections and attention;
  - K/Q projections for head-pair 0 run first (exp starts ~10us in);
    the V projection and head-pair-1 K/Q are interleaved into the first
    three attention chunks' windows;
  - AV runs in the transposed orientation (lhsT = expT tile, rhs = V_aug
    with a ones column) -> out[q, 65] with softmax denominators free;
    per-partition reciprocal+multiply normalizes, PE-transpose restores
    [dg, q] for the K=256-accumulated Wo matmul (single bf16 output).
"""

import sys

import numpy as np

if "/opt/trn_rl_repo" not in sys.path:
    sys.path.insert(0, "/opt/trn_rl_repo")

import ml_dtypes

import concourse.bacc as bacc
import concourse.mybir as mybir
import concourse.tile as tile
from concourse.masks import make_identity

F32 = mybir.dt.float32
BF16 = mybir.dt.bfloat16

B, S, D, H = 2, 2048, 1024, 16
DK = D // H          # 64
HL = 4               # heads per core
DG = HL * DK         # 256
SCALE = 0.125        # 1/sqrt(DK)

ET = D // 128        # 8 e-tiles (contraction over D)
JT = S // 128        # 16 j-tiles (keys)
QC = S // 512        # 4 q-chunks


def _build_nc():
    nc = bacc.Bacc("TRN2", target_bir_lowering=False, debug=False)

    xq = nc.dram_tensor("xq", [D, S], BF16, kind="ExternalInput").ap()
    xk = nc.dram_tensor("xk", [D, S], BF16, kind="ExternalInput").ap()
    xv = nc.dram_tensor("xv", [D, S], BF16, kind="ExternalInput").ap()
    wq = nc.dram_tensor("wq", [D, DG], BF16, kind="ExternalInput").ap()
    wk = nc.dram_tensor("wk", [D, DG], BF16, kind="ExternalInput").ap()
    wv = nc.dram_tensor("wv", [D, DG], BF16, kind="ExternalInput").ap()
    wo = nc.dram_tensor("wo", [DG, D], BF16, kind="ExternalInput").ap()
    out = nc.dram_tensor("out", [S, D], BF16, kind="ExternalOutput").ap()

    with tile.TileContext(nc) as tc:
        with (
            tc.tile_pool(name="wpool", bufs=1) as wpool,
            tc.tile_pool(name="xin", bufs=1) as xin,
            tc.tile_pool(name="proj", bufs=1) as proj,
            tc.tile_pool(name="expp", bufs=26) as expp,
            tc.tile_pool(name="nrm", bufs=6) as nrm,
            tc.tile_pool(name="osbp", bufs=2) as osbp,
            tc.tile_pool(name="ps", bufs=1, space="PSUM") as ps,
        ):
            # ---- ACT warmup: force the Exp table load at t=0 --------------
            wu_in = wpool.tile([128, 16], F32, tag="wu", name="wu_in")
            nc.vector.memset(wu_in, 0.0)
            wu_out = wpool.tile([128, 16], BF16, tag="wuo", name="wu_out")
            nc.scalar.activation(
                out=wu_out, in_=wu_in,
                func=mybir.ActivationFunctionType.Exp, scale=1.0,
            )

            # ---- constants + weight tiles ---------------------------------
            wk_sb = [wpool.tile([128, DG], BF16, tag=f"wk{e}", name=f"wk{e}")
                     for e in range(ET)]
            wq_sb = [wpool.tile([128, DG], BF16, tag=f"wq{e}", name=f"wq{e}")
                     for e in range(ET)]
            wv_sb = [wpool.tile([128, DG], BF16, tag=f"wv{e}", name=f"wv{e}")
                     for e in range(ET)]
            wo_sb = [wpool.tile([128, D], BF16, tag=f"wo{p}", name=f"wo{p}")
                     for p in range(2)]

            ident_f = wpool.tile([128, 128], F32, tag="ident_f", name="ident_f")
            make_identity(nc, ident_f)
            ident = wpool.tile([128, 128], BF16, tag="ident", name="ident")
            nc.vector.tensor_copy(ident, ident_f)

            # ---- persistent activation tiles ------------------------------
            kt_sb = [proj.tile([128, S], BF16, tag=f"kt{p}", name=f"kt{p}")
                     for p in range(2)]
            qt_sb = [proj.tile([128, S], BF16, tag=f"qt{p}", name=f"qt{p}")
                     for p in range(2)]
            v_sb = proj.tile([128, JT, HL, DK + 1], BF16, tag="v", name="v_sb")
            nc.vector.memset(v_sb[:, :, :, DK:DK + 1], 1.0)
            outt_sb = [proj.tile([128, S], BF16, tag=f"ot{p}", name=f"outt{p}")
                       for p in range(2)]

            # ---- DMA emission (sync+gpsimd queues round-robin) ------------
            # Order keeps exp(c0) fed from ~10us: wk, xk[c0], wq, xq[c0],
            # xk[c1], xk[c2], wv, xv[c0], xk[c3], then column-interleaved.
            queues = [nc.sync, nc.gpsimd]
            rr = [0]

            def dq(dst, src):
                queues[rr[0] % 2].dma_start(dst, src)
                rr[0] += 1

            xk_t = [xin.tile([128, S], BF16, tag=f"xk{e}", name=f"xk{e}")
                    for e in range(ET)]
            xq_t = [xin.tile([128, S], BF16, tag=f"xq{e}", name=f"xq{e}")
                    for e in range(ET)]
            xv_t = [xin.tile([128, S], BF16, tag=f"xv{e}", name=f"xv{e}")
                    for e in range(ET)]

            def dx(ts, dram, c):
                sl = slice(c * 512, (c + 1) * 512)
                for e in range(ET):
                    dq(ts[e][:, sl], dram[e * 128:(e + 1) * 128, sl])

            def dw(ts, dram):
                for e in range(ET):
                    dq(ts[e], dram[e * 128:(e + 1) * 128, :])

            dw(wk_sb, wk)
            dx(xk_t, xk, 0)
            dw(wq_sb, wq)
            dx(xq_t, xq, 0)
            dx(xk_t, xk, 1)
            dx(xk_t, xk, 2)
            dw(wv_sb, wv)
            dx(xv_t, xv, 0)
            dx(xk_t, xk, 3)
            dx(xv_t, xv, 1)
            dx(xq_t, xq, 1)
            dx(xv_t, xv, 2)
            dx(xq_t, xq, 2)
            dx(xv_t, xv, 3)
            dx(xq_t, xq, 3)
            for p in range(2):
                dq(wo_sb[p], wo[p * 128:(p + 1) * 128, :])

            # ---- projections ---------------------------------------------
            def kq_proj(p, cs, which="kq"):
                """K and/or Q projection for head-pair p over chunks cs.
                p==0 pairs K+Q in one 2-bank "sc" slot; p==1 uses single
                1-bank "acc" slots (interleaved into the attention flow)."""
                for c in cs:
                    csl = slice(c * 512, (c + 1) * 512)
                    if p == 0:
                        acct = ps.tile([128, 2, 512], F32, tag="sc",
                                       name=f"pa{c}")
                        pairs = ((0, wk_sb, xk_t, kt_sb), (1, wq_sb, xq_t, qt_sb))
                        for i, w_sb, x_t, dst in pairs:
                            for e in range(ET):
                                nc.tensor.matmul(
                                    acct[:, i, :], w_sb[e][:, 0:128],
                                    x_t[e][:, csl],
                                    start=(e == 0), stop=(e == ET - 1))
                            nc.vector.tensor_copy(dst[0][:, csl], acct[:, i, :])
                    else:
                        srcs = {"k": (wk_sb, xk_t, kt_sb),
                                "q": (wq_sb, xq_t, qt_sb)}
                        for nm in which:
                            w_sb, x_t, dst = srcs[nm]
                            acc = ps.tile([128, 512], F32, tag="acc",
                                          name=f"p1{nm}{c}")
                            for e in range(ET):
                                nc.tensor.matmul(
                                    acc, w_sb[e][:, 128:256], x_t[e][:, csl],
                                    start=(e == 0), stop=(e == ET - 1))
                            nc.vector.tensor_copy(dst[1][:, csl], acc)

            def v_proj():
                for jt in range(JT):
                    jsl = slice(jt * 128, (jt + 1) * 128)
                    accv = ps.tile([128, DG], F32, tag="acc", name=f"pv{jt}")
                    for e in range(ET):
                        nc.tensor.matmul(
                            accv, xv_t[e][:, jsl], wv_sb[e],
                            start=(e == 0), stop=(e == ET - 1))
                    nc.vector.tensor_copy(
                        v_sb[:, jt, :, 0:DK],
                        accv.rearrange("j (h d) -> j h d", h=HL))

            # ---- attention tail: AV + normalize + transpose (+ Wo) --------
            def tail(p, c, ex_tiles):
                hA, hB = 2 * p, 2 * p + 1
                for k in range(4):          # one 128-query tile per wave
                    qsl = slice(k * 128, (k + 1) * 128)
                    accA = ps.tile([128, DK + 1], F32, tag="acc",
                                   name=f"avA{p}{c}{k}")
                    accB = ps.tile([128, DK + 1], F32, tag="acc",
                                   name=f"avB{p}{c}{k}")
                    for jt in range(JT):
                        nc.tensor.matmul(
                            accA, ex_tiles[jt][:, 0, qsl], v_sb[:, jt, hA, :],
                            start=(jt == 0), stop=(jt == JT - 1))
                        nc.tensor.matmul(
                            accB, ex_tiles[jt][:, 1, qsl], v_sb[:, jt, hB, :],
                            start=(jt == 0), stop=(jt == JT - 1))
                    recA = nrm.tile([128, 1], F32, tag="rec", name=f"rA{p}{c}{k}")
                    recB = nrm.tile([128, 1], F32, tag="rec", name=f"rB{p}{c}{k}")
                    nc.vector.reciprocal(recA, accA[:, DK:DK + 1])
                    nc.vector.reciprocal(recB, accB[:, DK:DK + 1])
                    nt = nrm.tile([128, 2, DK], BF16, tag="nt", name=f"nt{p}{c}{k}")
                    nc.vector.tensor_scalar(
                        nt[:, 0, :], accA[:, 0:DK], recA, None,
                        mybir.AluOpType.mult)
                    nc.vector.tensor_scalar(
                        nt[:, 1, :], accB[:, 0:DK], recB, None,
                        mybir.AluOpType.mult)
                    pt = ps.tile([128, 128], BF16, tag="acc", name=f"pt{p}{c}{k}")
                    nc.tensor.transpose(pt, nt.rearrange("q h d -> q (h d)"),
                                        ident)
                    qg = c * 4 + k
                    nc.vector.tensor_copy(
                        outt_sb[p][:, qg * 128:(qg + 1) * 128], pt)
                if p == 1:
                    for k in range(4):
                        qg = c * 4 + k
                        osb = osbp.tile([128, D], BF16, tag="osb",
                                        name=f"osb{c}{k}")
                        for ch in range(2):
                            chsl = slice(ch * 512, (ch + 1) * 512)
                            acc = ps.tile([128, 512], F32, tag="acc",
                                          name=f"po{c}{k}{ch}")
                            nc.tensor.matmul(
                                acc, outt_sb[0][:, qg * 128:(qg + 1) * 128],
                                wo_sb[0][:, chsl], start=True, stop=False)
                            nc.tensor.matmul(
                                acc, outt_sb[1][:, qg * 128:(qg + 1) * 128],
                                wo_sb[1][:, chsl], start=False, stop=True)
                            nc.vector.tensor_copy(osb[:, chsl], acc)
                        nc.sync.dma_start(
                            out[qg * 128:(qg + 1) * 128, :], osb)

            # ---- main flow ------------------------------------------------
            kq_proj(0, range(QC))

            pending = None
            i = 0
            for p in range(2):
                for c in range(QC):
                    csl = slice(c * 512, (c + 1) * 512)
                    ex_tiles = []
                    for jt in range(JT):
                        jsl = slice(jt * 128, (jt + 1) * 128)
                        sc = ps.tile([128, 2, 512], F32, tag="sc",
                                     name=f"sc{p}{c}{jt}")
                        nc.tensor.matmul(
                            sc[:, 0, :], kt_sb[p][0:64, jsl],
                            qt_sb[p][0:64, csl],
                            start=True, stop=True, tile_position=(0, 0))
                        nc.tensor.matmul(
                            sc[:, 1, :], kt_sb[p][64:128, jsl],
                            qt_sb[p][64:128, csl],
                            start=True, stop=True, tile_position=(64, 0))
                        ex = expp.tile([128, 2, 512], BF16, tag="ex",
                                       name=f"ex{p}{c}{jt}")
                        nc.scalar.activation(
                            out=ex, in_=sc,
                            func=mybir.ActivationFunctionType.Exp,
                            scale=SCALE)
                        ex_tiles.append(ex)
                    # Late phase-A work rides the "acc" ring between chunks:
                    # emitted before each deferred tail so its accumulators
                    # get earlier ring slots.
                    if i == 1:
                        v_proj()
                    elif i == 2:
                        kq_proj(1, range(QC), which="k")
                    elif i == 3:
                        kq_proj(1, range(QC), which="q")
                    if pending is not None:
                        tail(*pending)
                    pending = (p, c, ex_tiles)
                    i += 1
            tail(*pending)

    nc.compile()
    return nc


_NC = None


def _get_nc():
    global _NC
    if _NC is None:
        _NC = _build_nc()
    return _NC


def make_in_maps(query, key, value, Wq, Wk, Wv, Wo):
    bf = ml_dtypes.bfloat16
    xqT = [np.ascontiguousarray(np.asarray(query[b], dtype=np.float32).T.astype(bf))
           for b in range(B)]
    xkT = [np.ascontiguousarray(np.asarray(key[b], dtype=np.float32).T.astype(bf))
           for b in range(B)]
    xvT = [np.ascontiguousarray(np.asarray(value[b], dtype=np.float32).T.astype(bf))
           for b in range(B)]
    Wq = np.asarray(Wq, dtype=np.float32)
    Wk = np.asarray(Wk, dtype=np.float32)
    Wv = np.asarray(Wv, dtype=np.float32)
    Wo = np.asarray(Wo, dtype=np.float32)

    in_maps = []
    for core in range(8):
        b, g = divmod(core, 4)
        sl = slice(g * DG, (g + 1) * DG)
        in_maps.append({
            "xq": xqT[b],
            "xk": xkT[b],
            "xv": xvT[b],
            "wq": np.ascontiguousarray(Wq[:, sl].astype(bf)),
            "wk": np.ascontiguousarray(Wk[:, sl].astype(bf)),
            "wv": np.ascontiguousarray(Wv[:, sl].astype(bf)),
            "wo": np.ascontiguousarray(Wo[sl, :].astype(bf)),
        })
    return in_maps


def combine_results(results):
    out = np.zeros((B, S, D), dtype=np.float32)
    for core in range(8):
        out[core // 4] += results[core]["out"].astype(np.float32)
    return out


def kernel(query, key, value, Wq, Wk, Wv, Wo, _trace=False):
    from concourse import bass_utils

    nc = _get_nc()
    in_maps = make_in_maps(query, key, value, Wq, Wk, Wv, Wo)
    r = bass_utils.run_bass_kernel_spmd(
        nc, in_maps, core_ids=list(range(8)), trace=_trace
    )
    kernel.last_results = r
    return combine_results(r.results)
